# revision 20
# baseline (speedup 1.0000x reference)
"""Trainium2 Bass kernel for DiffMultiHeadedAttention (differential attention).

Model (per reference):
    q = x @ Wq.T + bq                 [B,T,1024]
    k = ef @ Wk.T + bk                [B,N,1024]
    v = ef @ Wv.T + bv                [B,N,1024]
    lambda_full = exp(sum(lq1*lk1)) - exp(sum(lq2*lk2)) + 0.8
    att  = softmax(causal_mask(q_hh @ k_hh.T / sqrt(32)))   per 32 half-heads
    out_h = att[2h] - lambda_full * att[2h+1]  @ v_h        per 16 heads
B=4, T=N=1024, H=16 heads of 64, 2H=32 half-heads of 32.

Sharding over 8 cores: core c = (batch b = c//2, head-group hg = c%2).
Each core owns one batch element and 8 full heads (16 half-heads) and
computes out^T [512, 1024] for its (b, head-slice); the host transposes
and reassembles the full [4, 1024, 1024] output.

On-core dataflow (all fp32):
  - PE transposes x[b], ef[b] and the 512-row weight slices (fp32 has no
    DMA transpose) to get contraction-major layouts.
  - qT = (WqT).T @ xT per o-chunk, kT likewise from efT, v = efT.T @ WvT.
  - Attention in transposed layout: attT[n,t] = kT.T @ qT (K=32 row-tiled
    pairs), E = exp(attT/sqrt(32)) with triangular mask on the diagonal
    128x128 block, causally skipping fully-masked n-tiles.
  - av: outT_psum[65, t] += [v_h | ones].T @ E  accumulated over n-chunks;
    row 64 gives the softmax denominators for free.
  - Combine: out = P_pos/s_pos - lambda * P_neg/s_neg via batched DVE
    reciprocal + gpsimd partition-broadcast, DMA'd out as outT [512,1024].
"""

import math

import numpy as np

B, T, N, HIDDEN = 4, 1024, 1024, 1024
H, HEAD, HALF = 16, 64, 32
O = 512            # per-core hidden slice (8 heads * 64)
HPC = 8            # heads per core
LAMBDA_INIT = 0.8
SCALE = 1.0 / math.sqrt(HALF)
P = 128
IC = HIDDEN // P   # 8 contraction chunks
OC = O // P        # 4 output chunks of the projections
NT = N // P        # 8 n-tiles (keys)
NCORES = 8

_STATE = {}


def _build_nc():
    from contextlib import ExitStack

    import concourse.bacc as bacc
    import concourse.mybir as mybir
    import concourse.tile as tile
    from concourse.bass import ts
    from concourse.masks import make_identity

    f32 = mybir.dt.float32
    f32r = mybir.dt.float32r
    AF = mybir.ActivationFunctionType
    ALU = mybir.AluOpType

    nc = bacc.Bacc("TRN2", target_bir_lowering=False, debug=False)

    x_d = nc.dram_tensor("x", [T, HIDDEN], f32, kind="ExternalInput")
    ef_d = nc.dram_tensor("ef", [N, HIDDEN], f32, kind="ExternalInput")
    wq_d = nc.dram_tensor("wq", [O, HIDDEN], f32, kind="ExternalInput")
    wk_d = nc.dram_tensor("wk", [O, HIDDEN], f32, kind="ExternalInput")
    wv_d = nc.dram_tensor("wv", [O, HIDDEN], f32, kind="ExternalInput")
    bq_d = nc.dram_tensor("bq", [1, O], f32, kind="ExternalInput")
    bk_d = nc.dram_tensor("bk", [1, O], f32, kind="ExternalInput")
    bv_d = nc.dram_tensor("bv", [1, O], f32, kind="ExternalInput")
    lq1_d = nc.dram_tensor("lq1", [1, HALF], f32, kind="ExternalInput")
    lq2_d = nc.dram_tensor("lq2", [1, HALF], f32, kind="ExternalInput")
    lk1_d = nc.dram_tensor("lk1", [1, HALF], f32, kind="ExternalInput")
    lk2_d = nc.dram_tensor("lk2", [1, HALF], f32, kind="ExternalInput")
    outT_d = nc.dram_tensor("outT", [O, T], f32, kind="ExternalOutput")

    with tile.TileContext(nc) as tc:
        with ExitStack() as ctx:
            const = ctx.enter_context(tc.tile_pool(name="const", bufs=1))
            ident = const.tile([P, P], f32)
            make_identity(nc, ident)

            # ---- lambda_full (tiny, computed once) ----
            lam_in = const.tile([1, 4, HALF], f32)
            nc.sync.dma_start(lam_in[:, 0, :], lq1_d[:])
            nc.sync.dma_start(lam_in[:, 1, :], lk1_d[:])
            nc.sync.dma_start(lam_in[:, 2, :], lq2_d[:])
            nc.sync.dma_start(lam_in[:, 3, :], lk2_d[:])
            lam_tmp = const.tile([1, 2, HALF], f32)
            nc.vector.tensor_mul(lam_tmp[:, 0, :], lam_in[:, 0, :], lam_in[:, 1, :])
            nc.vector.tensor_mul(lam_tmp[:, 1, :], lam_in[:, 2, :], lam_in[:, 3, :])
            lam_s = const.tile([1, 2], f32)
            nc.vector.tensor_reduce(
                lam_s, lam_tmp, axis=mybir.AxisListType.X, op=ALU.add
            )
            lam_e = const.tile([1, 2], f32)
            nc.scalar.activation(lam_e, lam_s, AF.Exp)
            # lam_neg = -(e1 - e2 + 0.8) = e2 - e1 - 0.8
            lam_neg = const.tile([1, 1], f32)
            nc.vector.tensor_sub(lam_neg, lam_e[:, 1:2], lam_e[:, 0:1])
            nc.vector.tensor_scalar_add(lam_neg, lam_neg, -LAMBDA_INIT)
            lam_neg_b8 = const.tile([8, 1], f32)
            nc.gpsimd.partition_broadcast(lam_neg_b8, lam_neg)
            ones64 = const.tile([1, 64], f32)
            nc.vector.memset(ones64, 1.0)
            # 0/1 upper-triangular mask (keep t_local >= n_local), doubled
            # along a middle dim so one DVE mul masks both half-heads.
            tri2 = const.tile([P, 2, P], f32)
            nc.gpsimd.memset(tri2, 1.0)
            nc.gpsimd.affine_select(
                out=tri2,
                in_=tri2,
                compare_op=ALU.is_ge,
                fill=0.0,
                base=0,
                pattern=[[0, 2], [1, P]],
                channel_multiplier=-1,
            )

            # ---- biases ----
            bq_sb = const.tile([P, OC], f32)
            nc.sync.dma_start(bq_sb, bq_d[0].rearrange("(a p) -> p a", p=P))
            bk_sb = const.tile([P, OC], f32)
            nc.sync.dma_start(bk_sb, bk_d[0].rearrange("(a p) -> p a", p=P))
            bv_1 = const.tile([1, O], f32)
            nc.sync.dma_start(bv_1, bv_d[:])
            bvb = const.tile([P, O], f32)
            nc.gpsimd.partition_broadcast(bvb, bv_1)

            # ---- persistent projection outputs ----
            proj = ctx.enter_context(tc.tile_pool(name="proj", bufs=1))
            qT = proj.tile([P, OC, T], f32r)          # [d-part, oc, t]
            kT = proj.tile([P, OC, N], f32r)          # [d-part, oc, n]
            vaug = proj.tile([P, NT, HPC, HEAD + 1], f32r)  # [n-part, nt, h, d|1]
            ones8 = const.tile([P, HPC], f32)
            nc.vector.memset(ones8, 1.0)
            for nt_ in range(NT):
                # f32r memset fails ISA codegen; route through a DVE copy
                nc.vector.tensor_copy(
                    vaug[:, nt_, :, HEAD : HEAD + 1],
                    ones8[:, :].rearrange("p (a b) -> p a b", b=1),
                )

            # =============== phase 1: transposes + projections ===============
            with (
                tc.tile_pool(name="ph1", bufs=2) as ph1,
                tc.tile_pool(name="big", bufs=1) as big,
                tc.tile_pool(name="ps_tr", bufs=4, space="PSUM") as ps_tr,
                tc.tile_pool(name="ps_pj", bufs=4, space="PSUM") as ps_pj,
            ):
                xT = big.tile([P, IC, T], f32r)
                efT = big.tile([P, IC, N], f32r)

                def transpose_in(src_d, dstT, evict):
                    for tt in range(T // P):
                        nat = ph1.tile([P, HIDDEN], f32, tag="nat", name="nat")
                        nc.sync.dma_start(nat, src_d[ts(tt, P), :])
                        for g4 in range(IC // 4):
                            pst = ps_tr.tile([P, 4, P], f32, tag="tr", name="pst")
                            for k in range(4):
                                nc.tensor.transpose(
                                    pst[:, k, :], nat[:, ts(4 * g4 + k, P)], ident
                                )
                            dst = dstT[:, 4 * g4 : 4 * g4 + 4, ts(tt, P)]
                            if (tt + g4) % 2 == evict:
                                nc.vector.tensor_copy(dst, pst)
                            else:
                                nc.scalar.copy(dst, pst)

                # ef first: unblocks v and k projections (and attention) earliest
                transpose_in(ef_d, efT, 0)

                # v projection: v[n, o] = sum_ic efT[ic].T @ WvT[ic]  (+bias)
                wvT = big.tile([P, IC, O], f32r)
                for oc in range(OC):
                    wnat = ph1.tile([P, HIDDEN], f32, tag="nat", name="wvnat")
                    nc.sync.dma_start(wnat, wv_d[ts(oc, P), :])
                    for g4 in range(IC // 4):
                        pst = ps_tr.tile([P, 4, P], f32, tag="tr", name="pst")
                        for k in range(4):
                            nc.tensor.transpose(
                                pst[:, k, :], wnat[:, ts(4 * g4 + k, P)], ident
                            )
                        dst = wvT[:, 4 * g4 : 4 * g4 + 4, ts(oc, P)]
                        if g4 == 0:
                            nc.vector.tensor_copy(dst, pst)
                        else:
                            nc.scalar.copy(dst, pst)
                for nt_ in range(NT):
                    psj = ps_pj.tile([P, 512], f32, tag="pj", name="psv")
                    for ic in range(IC):
                        nc.tensor.matmul(
                            psj,
                            efT[:, ic, ts(nt_, P)],
                            wvT[:, ic, :],
                            start=(ic == 0),
                            stop=(ic == IC - 1),
                        )
                    nc.vector.tensor_add(
                        vaug[:, nt_, :, 0:HEAD],
                        psj[:].rearrange("p (h d) -> p h d", h=HPC),
                        bvb[:].rearrange("p (h d) -> p h d", h=HPC),
                    )

                transpose_in(x_d, xT, 1)

                # q/k projections per o-chunk: qT[o,t] = sum_ic WqT[ic].T @ xT[ic]
                for oc in range(OC):
                    for w_d, b_sb, actT, dstT in (
                        (wk_d, bk_sb, efT, kT),
                        (wq_d, bq_sb, xT, qT),
                    ):
                        wnat = ph1.tile([P, HIDDEN], f32, tag="nat", name="wnat")
                        nc.sync.dma_start(wnat, w_d[ts(oc, P), :])
                        wT = ph1.tile([P, IC, P], f32r, tag="wT", name="wT")
                        for g4 in range(IC // 4):
                            pst = ps_tr.tile([P, 4, P], f32, tag="tr", name="pst")
                            for k in range(4):
                                nc.tensor.transpose(
                                    pst[:, k, :], wnat[:, ts(4 * g4 + k, P)], ident
                                )
                            if g4 == 0:
                                nc.vector.tensor_copy(wT[:, 0:4, :], pst)
                            else:
                                nc.scalar.copy(wT[:, 4:8, :], pst)
                        for t2 in range(2):
                            psj = ps_pj.tile([P, 512], f32, tag="pj", name="psj")
                            for ic in range(IC):
                                nc.tensor.matmul(
                                    psj,
                                    wT[:, ic, :],
                                    actT[:, ic, ts(t2, 512)],
                                    start=(ic == 0),
                                    stop=(ic == IC - 1),
                                )
                            nc.vector.tensor_scalar_add(
                                dstT[:, oc, ts(t2, 512)], psj, b_sb[:, oc : oc + 1]
                            )

            # =============== phase 2: attention ===============
            acc_sb = ctx.enter_context(tc.tile_pool(name="acc_sb", bufs=1))
            # P65[:, h, s, t]: rows 0..63 = (E_s @ v_h).T, row 64 = sum_n E_s
            P65 = acc_sb.tile([65, HPC, 2, T], f32)
            S_sb = acc_sb.tile([40, T], f32)

            with (
                tc.tile_pool(name="att_sb", bufs=4) as att_sb,
                tc.tile_pool(name="ps_qk", bufs=2, space="PSUM") as ps_qk,
                tc.tile_pool(name="ps_av", bufs=2, space="PSUM") as ps_av,
            ):
                for oc in range(OC):
                    for j in range(2):
                        h = 2 * oc + j
                        for tcv in range(2):
                            avp = ps_av.tile(
                                [65, 2, 512], f32, tag="av", name=f"av{h}_{tcv}"
                            )
                            nis = range(4) if tcv == 0 else range(NT)
                            last = nis[-1]
                            # sweep 1: qk + exp for all n-tiles (uniform PE
                            # geometry back-to-back; E tiles persist in SBUF)
                            Es = {}
                            for nt_ in nis:
                                t0 = nt_ * P
                                cs = max(t0, 512 * tcv)
                                w = 512 * (tcv + 1) - cs
                                att_ps = ps_qk.tile(
                                    [P, 2, 512], f32, tag="qk", name="attps"
                                )
                                E = att_sb.tile(
                                    [P, 2, 512], f32r, tag="E", bufs=10, name="E"
                                )
                                Es[nt_] = (E, w)
                                for s in range(2):
                                    base = 64 * j + 32 * s
                                    nc.tensor.matmul(
                                        att_ps[:, s, :w],
                                        kT[base : base + 32, oc, ts(nt_, P)],
                                        qT[base : base + 32, oc, cs : cs + w],
                                        start=True,
                                        stop=True,
                                        tile_position=(96, 0) if base == 96 else None,
                                    )
                                nc.scalar.activation(
                                    E[:, :, :w], att_ps[:, :, :w], AF.Exp, scale=SCALE
                                )
                                if cs == t0:
                                    # diagonal block: keep t_local >= n_local
                                    nc.gpsimd.tensor_mul(
                                        E[:, :, 0:P], E[:, :, 0:P], tri2
                                    )
                            # sweep 2: av accumulation, s-major so PE geometry
                            # and psum bank stay fixed within each run
                            for s in range(2):
                                for nt_ in nis:
                                    E, w = Es[nt_]
                                    off = 512 - w
                                    nc.tensor.matmul(
                                        avp[:, s, off : off + w],
                                        vaug[:, nt_, h, :],
                                        E[:, s, :w],
                                        start=(nt_ == 0),
                                        stop=(nt_ == last),
                                    )
                            if tcv == 0:
                                nc.vector.tensor_copy(
                                    P65[:, h, :, ts(tcv, 512)], avp[:, :, :]
                                )
                            else:
                                nc.scalar.copy(
                                    P65[:, h, :, ts(tcv, 512)], avp[:, :, :]
                                )
                            for s in range(2):
                                nc.sync.dma_start(
                                    S_sb[32 * s + h : 32 * s + h + 1, ts(tcv, 512)],
                                    P65[64:65, h, s, ts(tcv, 512)],
                                )

            # ---- combine: out = P_pos/s_pos - lambda * P_neg/s_neg ----
            # R rows 0..7 = 1/s_pos per head, rows 8..15 = -lambda/s_neg.
            with (
                tc.tile_pool(name="cmb_sb", bufs=3) as cmb_sb,
                tc.tile_pool(name="ps_cmb", bufs=3, space="PSUM") as ps_cmb,
            ):
                R = acc_sb.tile([40, T], f32)
                nc.vector.reciprocal(R[0:8, :], S_sb[0:8, :])
                nc.vector.reciprocal(R[32:40, :], S_sb[32:40, :])
                nc.vector.tensor_scalar_mul(R[32:40, :], R[32:40, :], lam_neg_b8)
                for h in range(HPC):
                    for tcv in range(2):
                        # broadcast the two reciprocal rows across 64 partitions
                        # via a K=1 matmul with a ones stationary (rhs must sit
                        # on partition 0, so stage rows there with tiny DMAs).
                        R1h = cmb_sb.tile([1, 2, 512], f32, tag="R1h", name="R1h")
                        for s in range(2):
                            nc.sync.dma_start(
                                R1h[:, s, :],
                                R[32 * s + h : 32 * s + h + 1, ts(tcv, 512)],
                            )
                        if h % 2 == 0:
                            rb = ps_cmb.tile([64, 2, 512], f32, tag="rb", name="rb")
                            for s in range(2):
                                nc.tensor.matmul(
                                    rb[:, s, :],
                                    ones64,
                                    R1h[:, s, :],
                                    start=True,
                                    stop=True,
                                )
                        else:
                            rb = cmb_sb.tile([64, 2, 512], f32, tag="rbs", name="rbs")
                            for s in range(2):
                                nc.gpsimd.partition_broadcast(rb[:, s, :], R1h[:, s, :])
                        m = cmb_sb.tile([64, 2, 512], f32, tag="m", name="m")
                        nc.vector.tensor_mul(
                            m, P65[0:64, h, :, ts(tcv, 512)], rb[:, :, :]
                        )
                        nc.sync.dma_start(
                            outT_d[64 * h : 64 * h + 64, ts(tcv, 512)], m[:, 0, :]
                        )
                        nc.gpsimd.dma_start(
                            outT_d[64 * h : 64 * h + 64, ts(tcv, 512)],
                            m[:, 1, :],
                            accum_op=ALU.add,
                        )

    nc.compile()
    return nc


def _get_state():
    if "nc" not in _STATE:
        from concourse.bass_utils import run_bass_kernel_spmd

        _STATE["nc"] = _build_nc()
        _STATE["run"] = run_bass_kernel_spmd
    return _STATE


def kernel(**inputs):
    st = _get_state()

    def f32c(a):
        return np.ascontiguousarray(np.asarray(a, dtype=np.float32))

    x = f32c(inputs["x"])
    ef = f32c(inputs["encoder_feature"])
    Wq, bq = f32c(inputs["Wq"]), f32c(inputs["bq"])
    Wk, bk = f32c(inputs["Wk"]), f32c(inputs["bk"])
    Wv, bv = f32c(inputs["Wv"]), f32c(inputs["bv"])
    lq1 = f32c(inputs["lambda_q1"]).reshape(1, HALF)
    lq2 = f32c(inputs["lambda_q2"]).reshape(1, HALF)
    lk1 = f32c(inputs["lambda_k1"]).reshape(1, HALF)
    lk2 = f32c(inputs["lambda_k2"]).reshape(1, HALF)

    in_maps = []
    for c in range(NCORES):
        b, hg = c // 2, c % 2
        sl = slice(hg * O, (hg + 1) * O)
        in_maps.append(
            {
                "x": f32c(x[b]),
                "ef": f32c(ef[b]),
                "wq": f32c(Wq[sl]),
                "wk": f32c(Wk[sl]),
                "wv": f32c(Wv[sl]),
                "bq": f32c(bq[sl]).reshape(1, O),
                "bk": f32c(bk[sl]).reshape(1, O),
                "bv": f32c(bv[sl]).reshape(1, O),
                "lq1": lq1,
                "lq2": lq2,
                "lk1": lk1,
                "lk2": lk2,
            }
        )

    res = st["run"](st["nc"], in_maps, core_ids=list(range(NCORES)))
    _STATE["last_results"] = res

    out = np.empty((B, T, HIDDEN), dtype=np.float32)
    for c in range(NCORES):
        b, hg = c // 2, c % 2
        out[b, :, hg * O : (hg + 1) * O] = res.results[c]["outT"].T
    return out


# revision 21
# speedup vs baseline: 1.0230x; 1.0230x over previous
"""Trainium2 Bass kernel for DiffMultiHeadedAttention (differential attention).

Model (per reference):
    q = x @ Wq.T + bq                 [B,T,1024]
    k = ef @ Wk.T + bk                [B,N,1024]
    v = ef @ Wv.T + bv                [B,N,1024]
    lambda_full = exp(sum(lq1*lk1)) - exp(sum(lq2*lk2)) + 0.8
    att  = softmax(causal_mask(q_hh @ k_hh.T / sqrt(32)))   per 32 half-heads
    out_h = att[2h] - lambda_full * att[2h+1]  @ v_h        per 16 heads
B=4, T=N=1024, H=16 heads of 64, 2H=32 half-heads of 32.

Sharding over 8 cores: core c = (batch b = c//2, head-group hg = c%2).
Each core owns one batch element and 8 full heads (16 half-heads) and
computes out^T [512, 1024] for its (b, head-slice); the host transposes
and reassembles the full [4, 1024, 1024] output.

On-core dataflow (all fp32):
  - PE transposes x[b], ef[b] and the 512-row weight slices (fp32 has no
    DMA transpose) to get contraction-major layouts.
  - qT = (WqT).T @ xT per o-chunk, kT likewise from efT, v = efT.T @ WvT.
  - Attention in transposed layout: attT[n,t] = kT.T @ qT (K=32 row-tiled
    pairs), E = exp(attT/sqrt(32)) with triangular mask on the diagonal
    128x128 block, causally skipping fully-masked n-tiles.
  - av: outT_psum[65, t] += [v_h | ones].T @ E  accumulated over n-chunks;
    row 64 gives the softmax denominators for free.
  - Combine: out = P_pos/s_pos - lambda * P_neg/s_neg via batched DVE
    reciprocal + gpsimd partition-broadcast, DMA'd out as outT [512,1024].
"""

import math

import numpy as np

B, T, N, HIDDEN = 4, 1024, 1024, 1024
H, HEAD, HALF = 16, 64, 32
O = 512            # per-core hidden slice (8 heads * 64)
HPC = 8            # heads per core
LAMBDA_INIT = 0.8
SCALE = 1.0 / math.sqrt(HALF)
P = 128
IC = HIDDEN // P   # 8 contraction chunks
OC = O // P        # 4 output chunks of the projections
NT = N // P        # 8 n-tiles (keys)
NCORES = 8

_STATE = {}


def _build_nc():
    from contextlib import ExitStack

    import concourse.bacc as bacc
    import concourse.mybir as mybir
    import concourse.tile as tile
    from concourse.bass import ts
    from concourse.masks import make_identity

    f32 = mybir.dt.float32
    f32r = mybir.dt.float32r
    AF = mybir.ActivationFunctionType
    ALU = mybir.AluOpType

    nc = bacc.Bacc("TRN2", target_bir_lowering=False, debug=False)

    x_d = nc.dram_tensor("x", [T, HIDDEN], f32, kind="ExternalInput")
    ef_d = nc.dram_tensor("ef", [N, HIDDEN], f32, kind="ExternalInput")
    wq_d = nc.dram_tensor("wq", [O, HIDDEN], f32, kind="ExternalInput")
    wk_d = nc.dram_tensor("wk", [O, HIDDEN], f32, kind="ExternalInput")
    wv_d = nc.dram_tensor("wv", [O, HIDDEN], f32, kind="ExternalInput")
    bq_d = nc.dram_tensor("bq", [1, O], f32, kind="ExternalInput")
    bk_d = nc.dram_tensor("bk", [1, O], f32, kind="ExternalInput")
    bv_d = nc.dram_tensor("bv", [1, O], f32, kind="ExternalInput")
    lq1_d = nc.dram_tensor("lq1", [1, HALF], f32, kind="ExternalInput")
    lq2_d = nc.dram_tensor("lq2", [1, HALF], f32, kind="ExternalInput")
    lk1_d = nc.dram_tensor("lk1", [1, HALF], f32, kind="ExternalInput")
    lk2_d = nc.dram_tensor("lk2", [1, HALF], f32, kind="ExternalInput")
    outT_d = nc.dram_tensor("outT", [O, T], f32, kind="ExternalOutput")

    with tile.TileContext(nc) as tc:
        with ExitStack() as ctx:
            const = ctx.enter_context(tc.tile_pool(name="const", bufs=1))
            ident = const.tile([P, P], f32)
            make_identity(nc, ident)

            # ---- lambda_full (tiny, computed once) ----
            lam_in = const.tile([1, 4, HALF], f32)
            nc.sync.dma_start(lam_in[:, 0, :], lq1_d[:])
            nc.sync.dma_start(lam_in[:, 1, :], lk1_d[:])
            nc.sync.dma_start(lam_in[:, 2, :], lq2_d[:])
            nc.sync.dma_start(lam_in[:, 3, :], lk2_d[:])
            lam_tmp = const.tile([1, 2, HALF], f32)
            nc.vector.tensor_mul(lam_tmp[:, 0, :], lam_in[:, 0, :], lam_in[:, 1, :])
            nc.vector.tensor_mul(lam_tmp[:, 1, :], lam_in[:, 2, :], lam_in[:, 3, :])
            lam_s = const.tile([1, 2], f32)
            nc.vector.tensor_reduce(
                lam_s, lam_tmp, axis=mybir.AxisListType.X, op=ALU.add
            )
            lam_e = const.tile([1, 2], f32)
            nc.scalar.activation(lam_e, lam_s, AF.Exp)
            # lam_neg = -(e1 - e2 + 0.8) = e2 - e1 - 0.8
            lam_neg = const.tile([1, 1], f32)
            nc.vector.tensor_sub(lam_neg, lam_e[:, 1:2], lam_e[:, 0:1])
            nc.vector.tensor_scalar_add(lam_neg, lam_neg, -LAMBDA_INIT)
            lam_neg_b8 = const.tile([8, 1], f32)
            nc.gpsimd.partition_broadcast(lam_neg_b8, lam_neg)
            ones64 = const.tile([1, 64], f32)
            nc.vector.memset(ones64, 1.0)
            # 0/1 upper-triangular mask (keep t_local >= n_local), doubled
            # along a middle dim so one DVE mul masks both half-heads.
            tri2 = const.tile([P, 2, P], f32)
            nc.gpsimd.memset(tri2, 1.0)
            nc.gpsimd.affine_select(
                out=tri2,
                in_=tri2,
                compare_op=ALU.is_ge,
                fill=0.0,
                base=0,
                pattern=[[0, 2], [1, P]],
                channel_multiplier=-1,
            )

            # ---- biases ----
            bq_sb = const.tile([P, OC], f32)
            nc.sync.dma_start(bq_sb, bq_d[0].rearrange("(a p) -> p a", p=P))
            bk_sb = const.tile([P, OC], f32)
            nc.sync.dma_start(bk_sb, bk_d[0].rearrange("(a p) -> p a", p=P))
            bv_1 = const.tile([1, O], f32)
            nc.sync.dma_start(bv_1, bv_d[:])
            bvb = const.tile([P, O], f32)
            nc.gpsimd.partition_broadcast(bvb, bv_1)

            # ---- persistent projection outputs ----
            proj = ctx.enter_context(tc.tile_pool(name="proj", bufs=1))
            qT = proj.tile([P, OC, T], f32r)          # [d-part, oc, t]
            kT = proj.tile([P, OC, N], f32r)          # [d-part, oc, n]
            vaug = proj.tile([P, NT, HPC, HEAD + 1], f32r)  # [n-part, nt, h, d|1]
            ones8 = const.tile([P, HPC], f32)
            nc.vector.memset(ones8, 1.0)
            for nt_ in range(NT):
                # f32r memset fails ISA codegen; route through a DVE copy
                nc.vector.tensor_copy(
                    vaug[:, nt_, :, HEAD : HEAD + 1],
                    ones8[:, :].rearrange("p (a b) -> p a b", b=1),
                )

            # =============== phase 1: transposes + projections ===============
            with (
                tc.tile_pool(name="ph1", bufs=2) as ph1,
                tc.tile_pool(name="big", bufs=1) as big,
                tc.tile_pool(name="ps_tr", bufs=4, space="PSUM") as ps_tr,
                tc.tile_pool(name="ps_pj", bufs=4, space="PSUM") as ps_pj,
            ):
                xT = big.tile([P, IC, T], f32r)
                efT = big.tile([P, IC, N], f32r)

                def transpose_in(src_d, dstT, evict):
                    for tt in range(T // P):
                        nat = ph1.tile([P, HIDDEN], f32, tag="nat", name="nat")
                        nc.sync.dma_start(nat, src_d[ts(tt, P), :])
                        for g4 in range(IC // 4):
                            pst = ps_tr.tile([P, 4, P], f32, tag="tr", name="pst")
                            for k in range(4):
                                nc.tensor.transpose(
                                    pst[:, k, :], nat[:, ts(4 * g4 + k, P)], ident
                                )
                            dst = dstT[:, 4 * g4 : 4 * g4 + 4, ts(tt, P)]
                            if (tt + g4) % 2 == evict:
                                nc.vector.tensor_copy(dst, pst)
                            else:
                                nc.scalar.copy(dst, pst)

                # ef first: unblocks v and k projections (and attention) earliest
                transpose_in(ef_d, efT, 0)

                # v projection: v[n, o] = sum_ic efT[ic].T @ WvT[ic]  (+bias)
                wvT = big.tile([P, IC, O], f32r)
                for oc in range(OC):
                    wnat = ph1.tile([P, HIDDEN], f32, tag="nat", name="wvnat")
                    nc.sync.dma_start(wnat, wv_d[ts(oc, P), :])
                    for g4 in range(IC // 4):
                        pst = ps_tr.tile([P, 4, P], f32, tag="tr", name="pst")
                        for k in range(4):
                            nc.tensor.transpose(
                                pst[:, k, :], wnat[:, ts(4 * g4 + k, P)], ident
                            )
                        dst = wvT[:, 4 * g4 : 4 * g4 + 4, ts(oc, P)]
                        if g4 == 0:
                            nc.vector.tensor_copy(dst, pst)
                        else:
                            nc.scalar.copy(dst, pst)
                for nt_ in range(NT):
                    psj = ps_pj.tile([P, 512], f32, tag="pj", name="psv")
                    for ic in range(IC):
                        nc.tensor.matmul(
                            psj,
                            efT[:, ic, ts(nt_, P)],
                            wvT[:, ic, :],
                            start=(ic == 0),
                            stop=(ic == IC - 1),
                        )
                    nc.vector.tensor_add(
                        vaug[:, nt_, :, 0:HEAD],
                        psj[:].rearrange("p (h d) -> p h d", h=HPC),
                        bvb[:].rearrange("p (h d) -> p h d", h=HPC),
                    )

                transpose_in(x_d, xT, 1)

                # q/k projections per o-chunk: qT[o,t] = sum_ic WqT[ic].T @ xT[ic]
                for oc in range(OC):
                    for w_d, b_sb, actT, dstT in (
                        (wk_d, bk_sb, efT, kT),
                        (wq_d, bq_sb, xT, qT),
                    ):
                        wnat = ph1.tile([P, HIDDEN], f32, tag="nat", name="wnat")
                        nc.sync.dma_start(wnat, w_d[ts(oc, P), :])
                        wT = ph1.tile([P, IC, P], f32r, tag="wT", name="wT")
                        for g4 in range(IC // 4):
                            pst = ps_tr.tile([P, 4, P], f32, tag="tr", name="pst")
                            for k in range(4):
                                nc.tensor.transpose(
                                    pst[:, k, :], wnat[:, ts(4 * g4 + k, P)], ident
                                )
                            if g4 == 0:
                                nc.vector.tensor_copy(wT[:, 0:4, :], pst)
                            else:
                                nc.scalar.copy(wT[:, 4:8, :], pst)
                        for t2 in range(2):
                            psj = ps_pj.tile([P, 512], f32, tag="pj", name="psj")
                            for ic in range(IC):
                                nc.tensor.matmul(
                                    psj,
                                    wT[:, ic, :],
                                    actT[:, ic, ts(t2, 512)],
                                    start=(ic == 0),
                                    stop=(ic == IC - 1),
                                )
                            nc.vector.tensor_scalar_add(
                                dstT[:, oc, ts(t2, 512)], psj, b_sb[:, oc : oc + 1]
                            )

            # =============== phase 2: attention ===============
            acc_sb = ctx.enter_context(tc.tile_pool(name="acc_sb", bufs=1))
            # P65[:, h, s, t]: rows 0..63 = (E_s @ v_h).T, row 64 = sum_n E_s
            P65 = acc_sb.tile([65, HPC, 2, T], f32)
            S_sb = acc_sb.tile([40, T], f32)

            with (
                tc.tile_pool(name="att_sb", bufs=4) as att_sb,
                tc.tile_pool(name="ps_qk", bufs=2, space="PSUM") as ps_qk,
                tc.tile_pool(name="ps_av", bufs=2, space="PSUM") as ps_av,
            ):
                for oc in range(OC):
                    for j in range(2):
                        h = 2 * oc + j
                        for tcv in range(2):
                            avp = ps_av.tile(
                                [65, 2, 512], f32, tag="av", name=f"av{h}_{tcv}"
                            )
                            nis = range(4) if tcv == 0 else range(NT)
                            last = nis[-1]
                            # sweep 1: qk + exp for all n-tiles (uniform PE
                            # geometry back-to-back; E tiles persist in SBUF)
                            Es = {}
                            for nt_ in nis:
                                t0 = nt_ * P
                                cs = max(t0, 512 * tcv)
                                w = 512 * (tcv + 1) - cs
                                att_ps = ps_qk.tile(
                                    [P, 2, 512], f32, tag="qk", name="attps"
                                )
                                E = att_sb.tile(
                                    [P, 2, 512], f32r, tag="E", bufs=10, name="E"
                                )
                                Es[nt_] = (E, w)
                                for s in range(2):
                                    base = 64 * j + 32 * s
                                    nc.tensor.matmul(
                                        att_ps[:, s, :w],
                                        kT[base : base + 32, oc, ts(nt_, P)],
                                        qT[base : base + 32, oc, cs : cs + w],
                                        start=True,
                                        stop=True,
                                        tile_position=(96, 0) if base == 96 else None,
                                    )
                                nc.scalar.activation(
                                    E[:, :, :w], att_ps[:, :, :w], AF.Exp, scale=SCALE
                                )
                                if cs == t0:
                                    # diagonal block: keep t_local >= n_local
                                    nc.vector.tensor_mul(
                                        E[:, :, 0:P], E[:, :, 0:P], tri2
                                    )
                            # sweep 2: av accumulation, s-major so PE geometry
                            # and psum bank stay fixed within each run
                            for s in range(2):
                                for nt_ in nis:
                                    E, w = Es[nt_]
                                    off = 512 - w
                                    nc.tensor.matmul(
                                        avp[:, s, off : off + w],
                                        vaug[:, nt_, h, :],
                                        E[:, s, :w],
                                        start=(nt_ == 0),
                                        stop=(nt_ == last),
                                    )
                            if tcv == 0:
                                nc.vector.tensor_copy(
                                    P65[:, h, :, ts(tcv, 512)], avp[:, :, :]
                                )
                            else:
                                nc.scalar.copy(
                                    P65[:, h, :, ts(tcv, 512)], avp[:, :, :]
                                )
                            for s in range(2):
                                nc.sync.dma_start(
                                    S_sb[32 * s + h : 32 * s + h + 1, ts(tcv, 512)],
                                    P65[64:65, h, s, ts(tcv, 512)],
                                )

            # ---- combine: out = P_pos/s_pos - lambda * P_neg/s_neg ----
            # R rows 0..7 = 1/s_pos per head, rows 8..15 = -lambda/s_neg.
            with (
                tc.tile_pool(name="cmb_sb", bufs=3) as cmb_sb,
                tc.tile_pool(name="ps_cmb", bufs=3, space="PSUM") as ps_cmb,
            ):
                R = acc_sb.tile([40, T], f32)
                nc.vector.reciprocal(R[0:8, :], S_sb[0:8, :])
                nc.vector.reciprocal(R[32:40, :], S_sb[32:40, :])
                nc.vector.tensor_scalar_mul(R[32:40, :], R[32:40, :], lam_neg_b8)
                for h in range(HPC):
                    for tcv in range(2):
                        # broadcast the two reciprocal rows across 64 partitions
                        # via a K=1 matmul with a ones stationary (rhs must sit
                        # on partition 0, so stage rows there with tiny DMAs).
                        R1h = cmb_sb.tile([1, 2, 512], f32, tag="R1h", name="R1h")
                        for s in range(2):
                            nc.sync.dma_start(
                                R1h[:, s, :],
                                R[32 * s + h : 32 * s + h + 1, ts(tcv, 512)],
                            )
                        if h % 2 == 0:
                            rb = ps_cmb.tile([64, 2, 512], f32, tag="rb", name="rb")
                            for s in range(2):
                                nc.tensor.matmul(
                                    rb[:, s, :],
                                    ones64,
                                    R1h[:, s, :],
                                    start=True,
                                    stop=True,
                                )
                        else:
                            rb = cmb_sb.tile([64, 2, 512], f32, tag="rbs", name="rbs")
                            for s in range(2):
                                nc.gpsimd.partition_broadcast(rb[:, s, :], R1h[:, s, :])
                        m = cmb_sb.tile([64, 2, 512], f32, tag="m", name="m")
                        nc.vector.tensor_mul(
                            m, P65[0:64, h, :, ts(tcv, 512)], rb[:, :, :]
                        )
                        nc.sync.dma_start(
                            outT_d[64 * h : 64 * h + 64, ts(tcv, 512)], m[:, 0, :]
                        )
                        nc.gpsimd.dma_start(
                            outT_d[64 * h : 64 * h + 64, ts(tcv, 512)],
                            m[:, 1, :],
                            accum_op=ALU.add,
                        )

    nc.compile()
    return nc


def _get_state():
    if "nc" not in _STATE:
        from concourse.bass_utils import run_bass_kernel_spmd

        _STATE["nc"] = _build_nc()
        _STATE["run"] = run_bass_kernel_spmd
    return _STATE


def kernel(**inputs):
    st = _get_state()

    def f32c(a):
        return np.ascontiguousarray(np.asarray(a, dtype=np.float32))

    x = f32c(inputs["x"])
    ef = f32c(inputs["encoder_feature"])
    Wq, bq = f32c(inputs["Wq"]), f32c(inputs["bq"])
    Wk, bk = f32c(inputs["Wk"]), f32c(inputs["bk"])
    Wv, bv = f32c(inputs["Wv"]), f32c(inputs["bv"])
    lq1 = f32c(inputs["lambda_q1"]).reshape(1, HALF)
    lq2 = f32c(inputs["lambda_q2"]).reshape(1, HALF)
    lk1 = f32c(inputs["lambda_k1"]).reshape(1, HALF)
    lk2 = f32c(inputs["lambda_k2"]).reshape(1, HALF)

    in_maps = []
    for c in range(NCORES):
        b, hg = c // 2, c % 2
        sl = slice(hg * O, (hg + 1) * O)
        in_maps.append(
            {
                "x": f32c(x[b]),
                "ef": f32c(ef[b]),
                "wq": f32c(Wq[sl]),
                "wk": f32c(Wk[sl]),
                "wv": f32c(Wv[sl]),
                "bq": f32c(bq[sl]).reshape(1, O),
                "bk": f32c(bk[sl]).reshape(1, O),
                "bv": f32c(bv[sl]).reshape(1, O),
                "lq1": lq1,
                "lq2": lq2,
                "lk1": lk1,
                "lk2": lk2,
            }
        )

    res = st["run"](st["nc"], in_maps, core_ids=list(range(NCORES)))
    _STATE["last_results"] = res

    out = np.empty((B, T, HIDDEN), dtype=np.float32)
    for c in range(NCORES):
        b, hg = c // 2, c % 2
        out[b, :, hg * O : (hg + 1) * O] = res.results[c]["outT"].T
    return out


# revision 22
# speedup vs baseline: 1.0750x; 1.0508x over previous
"""Trainium2 Bass kernel for DiffMultiHeadedAttention (differential attention).

Model (per reference):
    q = x @ Wq.T + bq                 [B,T,1024]
    k = ef @ Wk.T + bk                [B,N,1024]
    v = ef @ Wv.T + bv                [B,N,1024]
    lambda_full = exp(sum(lq1*lk1)) - exp(sum(lq2*lk2)) + 0.8
    att  = softmax(causal_mask(q_hh @ k_hh.T / sqrt(32)))   per 32 half-heads
    out_h = att[2h] - lambda_full * att[2h+1]  @ v_h        per 16 heads
B=4, T=N=1024, H=16 heads of 64, 2H=32 half-heads of 32.

Sharding over 8 cores: core c = (batch b = c//2, head-group hg = c%2).
Each core owns one batch element and 8 full heads (16 half-heads) and
computes out^T [512, 1024] for its (b, head-slice); the host transposes
and reassembles the full [4, 1024, 1024] output.

On-core dataflow (all fp32):
  - PE transposes x[b], ef[b] and the 512-row weight slices (fp32 has no
    DMA transpose) to get contraction-major layouts.
  - qT = (WqT).T @ xT per o-chunk, kT likewise from efT, v = efT.T @ WvT.
  - Attention in transposed layout: attT[n,t] = kT.T @ qT (K=32 row-tiled
    pairs), E = exp(attT/sqrt(32)) with triangular mask on the diagonal
    128x128 block, causally skipping fully-masked n-tiles.
  - av: outT_psum[65, t] += [v_h | ones].T @ E  accumulated over n-chunks;
    row 64 gives the softmax denominators for free.
  - Combine: out = P_pos/s_pos - lambda * P_neg/s_neg via batched DVE
    reciprocal + gpsimd partition-broadcast, DMA'd out as outT [512,1024].
"""

import math

import numpy as np

B, T, N, HIDDEN = 4, 1024, 1024, 1024
H, HEAD, HALF = 16, 64, 32
O = 512            # per-core hidden slice (8 heads * 64)
HPC = 8            # heads per core
LAMBDA_INIT = 0.8
SCALE = 1.0 / math.sqrt(HALF)
P = 128
IC = HIDDEN // P   # 8 contraction chunks
OC = O // P        # 4 output chunks of the projections
NT = N // P        # 8 n-tiles (keys)
NCORES = 8

_STATE = {}


def _build_nc():
    from contextlib import ExitStack

    import concourse.bacc as bacc
    import concourse.mybir as mybir
    import concourse.tile as tile
    from concourse.bass import ts
    from concourse.masks import make_identity

    f32 = mybir.dt.float32
    f32r = mybir.dt.float32r
    AF = mybir.ActivationFunctionType
    ALU = mybir.AluOpType

    nc = bacc.Bacc("TRN2", target_bir_lowering=False, debug=False)

    x_d = nc.dram_tensor("x", [T, HIDDEN], f32, kind="ExternalInput")
    ef_d = nc.dram_tensor("ef", [N, HIDDEN], f32, kind="ExternalInput")
    wq_d = nc.dram_tensor("wq", [O, HIDDEN], f32, kind="ExternalInput")
    wk_d = nc.dram_tensor("wk", [O, HIDDEN], f32, kind="ExternalInput")
    wv_d = nc.dram_tensor("wv", [O, HIDDEN], f32, kind="ExternalInput")
    bq_d = nc.dram_tensor("bq", [1, O], f32, kind="ExternalInput")
    bk_d = nc.dram_tensor("bk", [1, O], f32, kind="ExternalInput")
    bv_d = nc.dram_tensor("bv", [1, O], f32, kind="ExternalInput")
    lq1_d = nc.dram_tensor("lq1", [1, HALF], f32, kind="ExternalInput")
    lq2_d = nc.dram_tensor("lq2", [1, HALF], f32, kind="ExternalInput")
    lk1_d = nc.dram_tensor("lk1", [1, HALF], f32, kind="ExternalInput")
    lk2_d = nc.dram_tensor("lk2", [1, HALF], f32, kind="ExternalInput")
    outT_d = nc.dram_tensor("outT", [O, T], f32, kind="ExternalOutput")

    with tile.TileContext(nc) as tc:
        with ExitStack() as ctx:
            const = ctx.enter_context(tc.tile_pool(name="const", bufs=1))
            ident = const.tile([P, P], f32)
            make_identity(nc, ident)

            # ---- lambda_full (tiny, computed once) ----
            lam_in = const.tile([1, 4, HALF], f32)
            nc.sync.dma_start(lam_in[:, 0, :], lq1_d[:])
            nc.sync.dma_start(lam_in[:, 1, :], lk1_d[:])
            nc.sync.dma_start(lam_in[:, 2, :], lq2_d[:])
            nc.sync.dma_start(lam_in[:, 3, :], lk2_d[:])
            lam_tmp = const.tile([1, 2, HALF], f32)
            nc.vector.tensor_mul(lam_tmp[:, 0, :], lam_in[:, 0, :], lam_in[:, 1, :])
            nc.vector.tensor_mul(lam_tmp[:, 1, :], lam_in[:, 2, :], lam_in[:, 3, :])
            lam_s = const.tile([1, 2], f32)
            nc.vector.tensor_reduce(
                lam_s, lam_tmp, axis=mybir.AxisListType.X, op=ALU.add
            )
            lam_e = const.tile([1, 2], f32)
            nc.scalar.activation(lam_e, lam_s, AF.Exp)
            # lam_neg = -(e1 - e2 + 0.8) = e2 - e1 - 0.8
            lam_neg = const.tile([1, 1], f32)
            nc.vector.tensor_sub(lam_neg, lam_e[:, 1:2], lam_e[:, 0:1])
            nc.vector.tensor_scalar_add(lam_neg, lam_neg, -LAMBDA_INIT)
            lam_neg_b8 = const.tile([8, 1], f32)
            nc.gpsimd.partition_broadcast(lam_neg_b8, lam_neg)
            ones64 = const.tile([1, 64], f32)
            nc.vector.memset(ones64, 1.0)
            # 0/1 upper-triangular mask (keep t_local >= n_local), doubled
            # along a middle dim so one DVE mul masks both half-heads.
            tri2 = const.tile([P, 2, P], f32)
            nc.gpsimd.memset(tri2, 1.0)
            nc.gpsimd.affine_select(
                out=tri2,
                in_=tri2,
                compare_op=ALU.is_ge,
                fill=0.0,
                base=0,
                pattern=[[0, 2], [1, P]],
                channel_multiplier=-1,
            )

            # ---- biases ----
            bq_sb = const.tile([P, OC], f32)
            nc.sync.dma_start(bq_sb, bq_d[0].rearrange("(a p) -> p a", p=P))
            bk_sb = const.tile([P, OC], f32)
            nc.sync.dma_start(bk_sb, bk_d[0].rearrange("(a p) -> p a", p=P))
            bv_1 = const.tile([1, O], f32)
            nc.sync.dma_start(bv_1, bv_d[:])
            bvb = const.tile([P, O], f32)
            nc.gpsimd.partition_broadcast(bvb, bv_1)

            # ---- persistent projection outputs ----
            proj = ctx.enter_context(tc.tile_pool(name="proj", bufs=1))
            qT = proj.tile([P, OC, T], f32r)          # [d-part, oc, t]
            kT = proj.tile([P, OC, N], f32r)          # [d-part, oc, n]
            vaug = proj.tile([P, NT, HPC, HEAD + 1], f32r)  # [n-part, nt, h, d|1]
            ones8 = const.tile([P, HPC], f32)
            nc.vector.memset(ones8, 1.0)
            for nt_ in range(NT):
                # f32r memset fails ISA codegen; route through a DVE copy
                nc.vector.tensor_copy(
                    vaug[:, nt_, :, HEAD : HEAD + 1],
                    ones8[:, :].rearrange("p (a b) -> p a b", b=1),
                )

            # =============== phase 1: transposes + projections ===============
            with (
                tc.tile_pool(name="ph1", bufs=2) as ph1,
                tc.tile_pool(name="big", bufs=1) as big,
                tc.tile_pool(name="ps_tr", bufs=4, space="PSUM") as ps_tr,
                tc.tile_pool(name="ps_pj", bufs=4, space="PSUM") as ps_pj,
            ):
                xT = big.tile([P, IC, T], f32r)
                efT = big.tile([P, IC, N], f32r)

                def transpose_in(src_d, dstT, evict):
                    for tt in range(T // P):
                        nat = ph1.tile([P, HIDDEN], f32, tag="nat", name="nat")
                        nc.sync.dma_start(nat, src_d[ts(tt, P), :])
                        for g4 in range(IC // 4):
                            pst = ps_tr.tile([P, 4, P], f32, tag="tr", name="pst")
                            for k in range(4):
                                nc.tensor.transpose(
                                    pst[:, k, :], nat[:, ts(4 * g4 + k, P)], ident
                                )
                            dst = dstT[:, 4 * g4 : 4 * g4 + 4, ts(tt, P)]
                            if (tt + g4) % 2 == evict:
                                nc.vector.tensor_copy(dst, pst)
                            else:
                                nc.scalar.copy(dst, pst)

                # ef first: unblocks v and k projections (and attention) earliest
                transpose_in(ef_d, efT, 0)

                # v projection: v[n, o] = sum_ic efT[ic].T @ WvT[ic]  (+bias)
                wvT = big.tile([P, IC, O], f32r)
                for oc in range(OC):
                    wnat = ph1.tile([P, HIDDEN], f32, tag="nat", name="wvnat")
                    nc.sync.dma_start(wnat, wv_d[ts(oc, P), :])
                    for g4 in range(IC // 4):
                        pst = ps_tr.tile([P, 4, P], f32, tag="tr", name="pst")
                        for k in range(4):
                            nc.tensor.transpose(
                                pst[:, k, :], wnat[:, ts(4 * g4 + k, P)], ident
                            )
                        dst = wvT[:, 4 * g4 : 4 * g4 + 4, ts(oc, P)]
                        if g4 == 0:
                            nc.vector.tensor_copy(dst, pst)
                        else:
                            nc.scalar.copy(dst, pst)
                for nt_ in range(NT):
                    psj = ps_pj.tile([P, 512], f32, tag="pj", name="psv")
                    for ic in range(IC):
                        nc.tensor.matmul(
                            psj,
                            efT[:, ic, ts(nt_, P)],
                            wvT[:, ic, :],
                            start=(ic == 0),
                            stop=(ic == IC - 1),
                        )
                    nc.vector.tensor_add(
                        vaug[:, nt_, :, 0:HEAD],
                        psj[:].rearrange("p (h d) -> p h d", h=HPC),
                        bvb[:].rearrange("p (h d) -> p h d", h=HPC),
                    )

                transpose_in(x_d, xT, 1)

                # q/k projections per o-chunk: qT[o,t] = sum_ic WqT[ic].T @ xT[ic]
                for oc in range(OC):
                    for w_d, b_sb, actT, dstT in (
                        (wk_d, bk_sb, efT, kT),
                        (wq_d, bq_sb, xT, qT),
                    ):
                        wnat = ph1.tile([P, HIDDEN], f32, tag="nat", name="wnat")
                        nc.sync.dma_start(wnat, w_d[ts(oc, P), :])
                        wT = ph1.tile([P, IC, P], f32r, tag="wT", name="wT")
                        for g4 in range(IC // 4):
                            pst = ps_tr.tile([P, 4, P], f32, tag="tr", name="pst")
                            for k in range(4):
                                nc.tensor.transpose(
                                    pst[:, k, :], wnat[:, ts(4 * g4 + k, P)], ident
                                )
                            if g4 == 0:
                                nc.vector.tensor_copy(wT[:, 0:4, :], pst)
                            else:
                                nc.scalar.copy(wT[:, 4:8, :], pst)
                        for t2 in range(2):
                            psj = ps_pj.tile([P, 512], f32, tag="pj", name="psj")
                            for ic in range(IC):
                                nc.tensor.matmul(
                                    psj,
                                    wT[:, ic, :],
                                    actT[:, ic, ts(t2, 512)],
                                    start=(ic == 0),
                                    stop=(ic == IC - 1),
                                )
                            nc.vector.tensor_scalar_add(
                                dstT[:, oc, ts(t2, 512)], psj, b_sb[:, oc : oc + 1]
                            )

            # =============== phase 2: attention ===============
            acc_sb = ctx.enter_context(tc.tile_pool(name="acc_sb", bufs=1))
            # P65[:, h, s, t]: rows 0..63 = (E_s @ v_h).T, row 64 = sum_n E_s
            P65 = acc_sb.tile([65, HPC, 2, T], f32)
            S_sb = acc_sb.tile([40, T], f32)

            with (
                tc.tile_pool(name="att_sb", bufs=4) as att_sb,
                tc.tile_pool(name="ps_qk", bufs=2, space="PSUM") as ps_qk,
                tc.tile_pool(name="ps_av", bufs=2, space="PSUM") as ps_av,
            ):
                for oc in range(OC):
                    for j in range(2):
                        h = 2 * oc + j
                        for tcv in range(2):
                            avp = ps_av.tile(
                                [65, 2, 512], f32, tag="av", name=f"av{h}_{tcv}"
                            )
                            nis = range(4) if tcv == 0 else range(NT)
                            last = nis[-1]
                            # sweep 1: qk + exp for all n-tiles (uniform PE
                            # geometry back-to-back; E tiles persist in SBUF)
                            Es = {}
                            for nt_ in nis:
                                t0 = nt_ * P
                                cs = max(t0, 512 * tcv)
                                w = 512 * (tcv + 1) - cs
                                att_ps = ps_qk.tile(
                                    [P, 2, 512], f32, tag="qk", name="attps"
                                )
                                E = att_sb.tile(
                                    [P, 2, 512], f32r, tag="E", bufs=10, name="E"
                                )
                                Es[nt_] = (E, w)
                                for s in range(2):
                                    base = 64 * j + 32 * s
                                    nc.tensor.matmul(
                                        att_ps[:, s, :w],
                                        kT[base : base + 32, oc, ts(nt_, P)],
                                        qT[base : base + 32, oc, cs : cs + w],
                                        start=True,
                                        stop=True,
                                        tile_position=(96, 0) if base == 96 else None,
                                    )
                                nc.scalar.activation(
                                    E[:, :, :w], att_ps[:, :, :w], AF.Exp, scale=SCALE
                                )
                                if cs == t0:
                                    # diagonal block: keep t_local >= n_local
                                    nc.vector.tensor_mul(
                                        E[:, :, 0:P], E[:, :, 0:P], tri2
                                    )
                            # sweep 2: av accumulation, s-major so PE geometry
                            # and psum bank stay fixed within each run
                            for s in range(2):
                                for nt_ in nis:
                                    E, w = Es[nt_]
                                    off = 512 - w
                                    nc.tensor.matmul(
                                        avp[:, s, off : off + w],
                                        vaug[:, nt_, h, :],
                                        E[:, s, :w],
                                        start=(nt_ == 0),
                                        stop=(nt_ == last),
                                    )
                            nc.vector.tensor_copy(
                                P65[:, h, :, ts(tcv, 512)], avp[:, :, :]
                            )
                            for s in range(2):
                                nc.sync.dma_start(
                                    S_sb[32 * s + h : 32 * s + h + 1, ts(tcv, 512)],
                                    P65[64:65, h, s, ts(tcv, 512)],
                                )

            # ---- combine: out = P_pos/s_pos - lambda * P_neg/s_neg ----
            # R rows 0..7 = 1/s_pos per head, rows 8..15 = -lambda/s_neg.
            with (
                tc.tile_pool(name="cmb_sb", bufs=3) as cmb_sb,
                tc.tile_pool(name="ps_cmb", bufs=3, space="PSUM") as ps_cmb,
            ):
                R = acc_sb.tile([40, T], f32)
                nc.vector.reciprocal(R[0:8, :], S_sb[0:8, :])
                nc.vector.reciprocal(R[32:40, :], S_sb[32:40, :])
                nc.vector.tensor_scalar_mul(R[32:40, :], R[32:40, :], lam_neg_b8)
                for h in range(HPC):
                    for tcv in range(2):
                        # broadcast the two reciprocal rows across 64 partitions
                        # via a K=1 matmul with a ones stationary (rhs must sit
                        # on partition 0, so stage rows there with tiny DMAs).
                        R1h = cmb_sb.tile([1, 2, 512], f32, tag="R1h", name="R1h")
                        for s in range(2):
                            nc.sync.dma_start(
                                R1h[:, s, :],
                                R[32 * s + h : 32 * s + h + 1, ts(tcv, 512)],
                            )
                        if h % 2 == 0:
                            rb = ps_cmb.tile([64, 2, 512], f32, tag="rb", name="rb")
                            for s in range(2):
                                nc.tensor.matmul(
                                    rb[:, s, :],
                                    ones64,
                                    R1h[:, s, :],
                                    start=True,
                                    stop=True,
                                )
                        else:
                            rb = cmb_sb.tile([64, 2, 512], f32, tag="rbs", name="rbs")
                            for s in range(2):
                                nc.gpsimd.partition_broadcast(rb[:, s, :], R1h[:, s, :])
                        m = cmb_sb.tile([64, 2, 512], f32, tag="m", name="m")
                        nc.vector.tensor_mul(
                            m, P65[0:64, h, :, ts(tcv, 512)], rb[:, :, :]
                        )
                        nc.sync.dma_start(
                            outT_d[64 * h : 64 * h + 64, ts(tcv, 512)], m[:, 0, :]
                        )
                        nc.gpsimd.dma_start(
                            outT_d[64 * h : 64 * h + 64, ts(tcv, 512)],
                            m[:, 1, :],
                            accum_op=ALU.add,
                        )

    nc.compile()
    return nc


def _get_state():
    if "nc" not in _STATE:
        from concourse.bass_utils import run_bass_kernel_spmd

        _STATE["nc"] = _build_nc()
        _STATE["run"] = run_bass_kernel_spmd
    return _STATE


def kernel(**inputs):
    st = _get_state()

    def f32c(a):
        return np.ascontiguousarray(np.asarray(a, dtype=np.float32))

    x = f32c(inputs["x"])
    ef = f32c(inputs["encoder_feature"])
    Wq, bq = f32c(inputs["Wq"]), f32c(inputs["bq"])
    Wk, bk = f32c(inputs["Wk"]), f32c(inputs["bk"])
    Wv, bv = f32c(inputs["Wv"]), f32c(inputs["bv"])
    lq1 = f32c(inputs["lambda_q1"]).reshape(1, HALF)
    lq2 = f32c(inputs["lambda_q2"]).reshape(1, HALF)
    lk1 = f32c(inputs["lambda_k1"]).reshape(1, HALF)
    lk2 = f32c(inputs["lambda_k2"]).reshape(1, HALF)

    in_maps = []
    for c in range(NCORES):
        b, hg = c // 2, c % 2
        sl = slice(hg * O, (hg + 1) * O)
        in_maps.append(
            {
                "x": f32c(x[b]),
                "ef": f32c(ef[b]),
                "wq": f32c(Wq[sl]),
                "wk": f32c(Wk[sl]),
                "wv": f32c(Wv[sl]),
                "bq": f32c(bq[sl]).reshape(1, O),
                "bk": f32c(bk[sl]).reshape(1, O),
                "bv": f32c(bv[sl]).reshape(1, O),
                "lq1": lq1,
                "lq2": lq2,
                "lk1": lk1,
                "lk2": lk2,
            }
        )

    res = st["run"](st["nc"], in_maps, core_ids=list(range(NCORES)))
    _STATE["last_results"] = res

    out = np.empty((B, T, HIDDEN), dtype=np.float32)
    for c in range(NCORES):
        b, hg = c // 2, c % 2
        out[b, :, hg * O : (hg + 1) * O] = res.results[c]["outT"].T
    return out
